# revision 26
# baseline (speedup 1.0000x reference)
"""Trainium2 Bass kernel for nn_DeformableAttention (B=4, C=384, H=W=56, NH=12, HC=32, STRIDE=2).

Self-contained: hardcodes shapes/sharding. Sharding: 8 cores = 4 batches x 2
pixel-row-halves. Each core computes the full value/key/offset branches for its
batch (duplicated across the pair) and the query branch + final GEMM for its
half of the 3136 output pixels.

Math note: the reference computes out = (scale * q^T k) v^T without softmax, so
attention is linear and reassociates:
    y[b] = (w_out @ blockdiag_h(scale * M[b,h])) @ Q[b],
    M[b,h] = V_s[b,h] K[b,h]^T  (32x32 per head)
which drops the 48x(3136x784x32) einsums to a few small GEMMs.

Measured HW model (trace-derived): DVE ~1.1ns/elem regardless of dtype/stride;
PE matmul ~M+128 cycles; Pool elementwise ~2ns/elem with a one-time ~60us
ucode-load on the first op (pre-warmed with dummies); per-DMA-queue bandwidth
~90GB/s (big loads are split into slices to ride multiple queues).

Schedule: value conv runs on the PE as 9 diag(w_tap) matmuls PSUM-accumulated
per 448-pixel chunk (bias folded into the PSUM->SBUF copy), freeing ~92us of
DVE. The off branch is fp32 end-to-end (sample positions are precision
critical). DVE keeps off/key/query convs + LN pointwise + bilinear. Gathers
fetch (x0,x0+1) pixel pairs as one 768-elem row. M accumulates over the 7
k-tiles directly in PSUM. floor() is computed via round(x-0.5) (casts round to
nearest; integer ties land on the complementary-weight corner, which is exact).
"""
import contextlib

import numpy as np

import concourse.bass as bass
import concourse.tile as tile
from concourse import bacc, mybir
from concourse.bass_utils import run_bass_kernel_spmd
from concourse.masks import make_identity

F32, F16, I32 = mybir.dt.float32, mybir.dt.float16, mybir.dt.int32
MULT, ADD, SUB = mybir.AluOpType.mult, mybir.AluOpType.add, mybir.AluOpType.subtract
AF = mybir.ActivationFunctionType

B, C, H, W = 4, 384, 56, 56
NH, HC = 12, 32
SCALE = HC ** -0.5
HP = H + 2                      # 58 padded
PIX = H * W                     # 3136
KH = KW = 28                    # stride-2 output
N = KH * KW                     # 784
NT = 112                        # point-tile size (7 tiles)
NTILES = N // NT
HALF_ROWS = H // 2              # 28
HALF_PIX = HALF_ROWS * W        # 1568
CT = C // 128                   # 3 channel tiles
EPS = 1e-5
VCH = 448                       # value-conv PE chunk (8 rows of 56)
NVCH = PIX // VCH               # 7 chunks per ct

_CACHE = {}


def _emit(nc, tc, ctx, io):
    pool = ctx.enter_context(tc.tile_pool(name="main", bufs=1))
    dma = nc.sync
    dma2 = nc.scalar
    gp = nc.gpsimd

    # ---------------- loads (big tensors split across DMA queues) ----------------
    def load_split(name, width, dtype, nsplit, eng):
        out = []
        for ct in range(CT):
            t = pool.tile([128, width], dtype, tag=f"{name}_{ct}")
            step = 128 // nsplit
            for s in range(nsplit):
                r0 = s * step
                eng.dma_start(t[r0:r0 + step, :],
                              io[name][ct * 128 + r0:ct * 128 + r0 + step, :])
            out.append(t)
        return out

    def load_cols(name, width, dtype=F32, eng=dma):
        out = []
        for ct in range(CT):
            t = pool.tile([128, width], dtype, tag=f"{name}_{ct}")
            eng.dma_start(t[:], io[name][ct * 128:(ct + 1) * 128, :])
            out.append(t)
        return out

    xp16 = load_split("xp16", HP * HP, F16, 4, dma)
    wv = load_cols("wv", 9)
    bv = load_cols("bv", 1)
    wo = load_cols("wo", 9)
    bo = load_cols("bo", 1)
    wk = load_cols("wk", 9)
    bk = load_cols("bk", 1)
    xq16 = load_split("xq", 30 * HP, F16, 2, dma)
    wq = load_cols("wq", 9)
    bq = load_cols("bq", 1)
    lng = load_cols("lng", 1)
    lnb = load_cols("lnb", 1)
    w2t32 = load_cols("w2t", 2)
    wot16 = load_cols("wot", C, dtype=F16, eng=dma)
    refyx = pool.tile([2, N], F32, tag="refyx")
    dma.dma_start(refyx[:], io["refyx"][:, :])
    ones_rc = pool.tile([128, 1], F16, tag="ones_rc")
    nc.vector.memset(ones_rc[:], 1.0 / C)
    one_row = pool.tile([1, 128], F16, tag="one_row")
    nc.vector.memset(one_row[:], 1.0)
    ident = pool.tile([128, 128], F16, tag="ident")
    make_identity(nc, ident[:])
    eps_t = pool.tile([1, 1], F32, tag="eps_t")
    nc.vector.memset(eps_t[:], EPS)

    # diag(w_tap) tiles (scalar engine: per-partition scale of the identity)
    def make_diag(w, nm):
        out = []
        for ct in range(CT):
            dd = []
            for t in range(9):
                d = pool.tile([128, 128], F16, tag=f"dg_{nm}_{ct}_{t}",
                              name=f"dg_{nm}_{ct}_{t}")
                nc.scalar.activation(d[:], ident[:], AF.Copy, scale=w[ct][:, t:t + 1])
                dd.append(d)
            out.append(dd)
        return out

    diag_v = make_diag(wv, "v")

    # ---------------- off conv (DVE, fp32 accumulate) ----------------
    off = []
    for ct in range(CT):
        t = pool.tile([128, N], F32, tag=f"off_{ct}")
        x3 = xp16[ct][:].rearrange("p (h w) -> p h w", h=HP)
        for tap in range(9):
            dy, dx = tap // 3, tap % 3
            src = x3[:, dy:dy + 2 * KH - 1:2, dx:dx + 2 * KW - 1:2]
            if tap == 0:
                nc.vector.tensor_scalar(out=t[:].rearrange("p (h w) -> p h w", h=KH),
                                        in0=src, scalar1=wo[ct][:, 0:1],
                                        scalar2=bo[ct][:, 0:1], op0=MULT, op1=ADD)
            else:
                nc.vector.scalar_tensor_tensor(
                    out=t[:].rearrange("p (h w) -> p h w", h=KH), in0=src,
                    scalar=wo[ct][:, tap:tap + 1],
                    in1=t[:].rearrange("p (h w) -> p h w", h=KH), op0=MULT, op1=ADD)
        off.append(t)

    # f16 copies for the LN-stats matmuls (stats precision is not critical)
    off16, sq16 = [], []
    for ct in range(CT):
        t = pool.tile([128, N], F16, tag=f"off16_{ct}")
        nc.scalar.activation(t[:], off[ct][:], AF.Copy)
        off16.append(t)
        s = pool.tile([128, N], F16, tag=f"sq16_{ct}")
        nc.scalar.activation(s[:], off[ct][:], AF.Square)
        sq16.append(s)

    # ---------------- value conv (PE diag-matmuls) + vtab transposes ----------
    val = [pool.tile([128, PIX], F16, tag=f"val_{ct}", name=f"val_{ct}")
           for ct in range(CT)]
    vps_ctx = tc.tile_pool(name="vps", bufs=2, space="PSUM")
    vps = vps_ctx.__enter__()
    vtctx = tc.tile_pool(name="vtp_ps", bufs=2, space="PSUM")
    vtps = vtctx.__enter__()
    vtsctx = tc.tile_pool(name="vtp_sb", bufs=3)
    vtsb = vtsctx.__enter__()
    vtab_writes = []

    def value_ct(ct):
        x3 = xp16[ct][:].rearrange("p (h w) -> p h w", h=HP)
        for chk in range(NVCH):
            r0 = chk * 8  # output row base of this 448-pixel chunk
            ps = vps.tile([128, VCH], F32, tag="vch", space="PSUM")
            for t in range(9):
                dy, dx = t // 3, t % 3
                src = x3[:, r0 + dy:r0 + dy + 8, dx:dx + W]
                nc.tensor.matmul(ps[:].rearrange("p (h w) -> p h w", h=8),
                                 diag_v[ct][t], src,
                                 start=(t == 0), stop=(t == 8),
                                 skip_group_check=True)
            nc.scalar.activation(val[ct][:, chk * VCH:(chk + 1) * VCH], ps[:],
                                 AF.Identity, bias=bv[ct][:, 0:1])

    def vtab_band(ct):
        for g in range(6):
            c0 = g * 512
            tp4 = vtps.tile([128, 512], F16, tag="tp4", space="PSUM")
            for j in range(4):
                nc.tensor.transpose(tp4[:, j * 128:(j + 1) * 128],
                                    val[ct][:, c0 + j * 128:c0 + (j + 1) * 128],
                                    ident[:])
            w4 = vtsb.tile([128, 512], F16, tag="w4")
            nc.scalar.activation(w4[:], tp4[:], AF.Copy)
            out_ap = bass.AP(io["vtab"].tensor, c0 * C + ct * 128,
                             [[C, 128], [C * 128, 4], [1, 128]])
            vtab_writes.append(dma2.dma_start(out_ap, w4[:]))
        tp1 = vtps.tile([64, 128], F16, tag="tp1", space="PSUM")
        nc.tensor.transpose(tp1[:], val[ct][:, 3072:3136], ident[:])
        w1 = vtsb.tile([64, 128], F16, tag="w1")
        nc.scalar.activation(w1[:], tp1[:], AF.Copy)
        vtab_writes.append(
            dma2.dma_start(io["vtab"][3072:3136, ct * 128:(ct + 1) * 128], w1[:]))

    value_ct(0)
    vtab_band(0)
    value_ct(1)
    vtab_band(1)

    # ---------------- LN stats (PE f16) + pointwise ----------------
    musq = pool.tile([1, N], F32, tag="musq")
    var = pool.tile([1, N], F32, tag="var")
    sd32 = pool.tile([1, N], F32, tag="sd32")
    rstd32 = pool.tile([1, N], F32, tag="rstd32")
    mu16 = pool.tile([1, N], F16, tag="mu16")
    rstd16 = pool.tile([1, N], F16, tag="rstd16")
    mu_b = pool.tile([128, N], F32, tag="mu_b")
    rstd_b = pool.tile([128, N], F32, tag="rstd_b")
    with tc.tile_pool(name="ln_psum", bufs=1, space="PSUM") as lnp:
        st_ps = lnp.tile([1, N], F32, tag="st_ps")
        for sl in (slice(0, 512), slice(512, N)):
            for ct in range(CT):
                nc.tensor.matmul(st_ps[:, sl], ones_rc[:], off16[ct][:, sl],
                                 start=(ct == 0), stop=(ct == CT - 1))
        nc.scalar.activation(musq[:], st_ps[:], AF.Square)
        nc.scalar.activation(mu16[:], st_ps[:], AF.Copy)
        for sl in (slice(0, 512), slice(512, N)):
            for ct in range(CT):
                nc.tensor.matmul(st_ps[:, sl], ones_rc[:], sq16[ct][:, sl],
                                 start=(ct == 0), stop=(ct == CT - 1))
        nc.vector.tensor_tensor(out=var[:], in0=st_ps[:], in1=musq[:], op=SUB)
    nc.scalar.activation(sd32[:], var[:], AF.Sqrt, bias=eps_t[:, 0:1])
    nc.vector.reciprocal_approx_fast(rstd32[:], sd32[:])
    nc.scalar.activation(rstd16[:], rstd32[:], AF.Copy)
    with tc.tile_pool(name="bc_psum", bufs=1, space="PSUM") as bcp:
        bc_ps = bcp.tile([128, N], F32, tag="bc_ps")
        for sl in (slice(0, 512), slice(512, N)):
            nc.tensor.matmul(bc_ps[:, sl], one_row[:], mu16[:, sl],
                             start=True, stop=True)
        nc.scalar.activation(mu_b[:], bc_ps[:], AF.Copy)
        for sl in (slice(0, 512), slice(512, N)):
            nc.tensor.matmul(bc_ps[:, sl], one_row[:], rstd16[:, sl],
                             start=True, stop=True)
        nc.scalar.activation(rstd_b[:], bc_ps[:], AF.Copy)

    # normalize (DVE, fp32) + gelu (scalar, g/b folded into activation)
    gel = []
    for ct in range(CT):
        t1 = off[ct]  # in-place
        nc.vector.tensor_tensor(out=t1[:], in0=t1[:], in1=mu_b[:], op=SUB)
        nc.vector.tensor_tensor(out=t1[:], in0=t1[:], in1=rstd_b[:], op=MULT)
        g = pool.tile([128, N], F32, tag=f"gel_{ct}")
        nc.scalar.activation(g[:], t1[:], AF.Gelu,
                             scale=lng[ct][:, 0:1], bias=lnb[ct][:, 0:1])
        gel.append(g)

    # ---------------- key conv (DVE, strided taps) ----------------
    key = []
    for ct in range(CT):
        t = pool.tile([128, N], F16, tag=f"key_{ct}")
        x3 = xp16[ct][:].rearrange("p (h w) -> p h w", h=HP)
        for tap in range(9):
            dy, dx = tap // 3, tap % 3
            src2 = x3[:, dy:dy + 2 * KH - 1:2, dx:dx + 2 * KW - 1:2]
            o2 = t[:].rearrange("p (h w) -> p h w", h=KH)
            if tap == 0:
                nc.vector.tensor_scalar(out=o2, in0=src2, scalar1=wk[ct][:, 0:1],
                                        scalar2=bk[ct][:, 0:1], op0=MULT, op1=ADD)
            else:
                nc.vector.scalar_tensor_tensor(out=o2, in0=src2,
                                               scalar=wk[ct][:, tap:tap + 1],
                                               in1=o2, op0=MULT, op1=ADD)
        key.append(t)


    # ---------------- offset head: w2t matmul (fp32) + tanh + ixy ----------------
    pos = pool.tile([2, N], F32, tag="pos")
    tnh = pool.tile([2, N], F32, tag="tnh")
    ixy0 = pool.tile([2, N], F32, tag="ixy0")
    with tc.tile_pool(name="off_psum", bufs=1, space="PSUM") as offp:
        oyx_ps = offp.tile([2, N], F32, tag="oyx")
        for sl in (slice(0, 512), slice(512, N)):
            for ct in range(CT):
                nc.tensor.matmul(oyx_ps[:, sl], w2t32[ct][:], gel[ct][:, sl],
                                 start=(ct == 0), stop=(ct == CT - 1))
        oyx_sb = pool.tile([2, N], F32, tag="oyx_sb")
        nc.scalar.activation(oyx_sb[:], oyx_ps[:], AF.Copy)

    value_ct(2)
    vtab_band(2)
    vtsctx.__exit__(None, None, None)
    vtctx.__exit__(None, None, None)
    vps_ctx.__exit__(None, None, None)

    nc.vector.tensor_tensor(out=pos[:], in0=oyx_sb[:], in1=refyx[:], op=ADD)
    nc.scalar.activation(tnh[:], pos[:], AF.Tanh)
    # iy/ix - 0.5 = tanh*27.5 + 27.0  (the -0.5 shift makes round() act as floor)
    nc.vector.tensor_scalar(out=ixy0[:], in0=tnh[:], scalar1=(H - 1) / 2.0,
                            scalar2=(H - 1) / 2.0 - 0.5, op0=MULT, op1=ADD)
    ixy_write = dma.dma_start(io["ixy_dram"][:, :], ixy0[:])

    # ---------------- index math (DVE, wide tiles) ----------------
    # layout [112 pts, 14]: cols 0..6 = iy-0.5 per k-tile, cols 7..13 = ix-0.5
    iyx = pool.tile([NT, 2 * NTILES], F32, tag="iyx")
    for j in range(2):
        src = bass.AP(io["ixy_dram"].tensor, j * N, [[1, NT], [NT, NTILES]])
        rd = dma.dma_start(iyx[:, j * NTILES:(j + 1) * NTILES], src)
        tile.add_dep_helper(rd.ins, ixy_write.ins, reason="ixy dram RAW")
    x0i = pool.tile([NT, 2 * NTILES], I32, tag="x0i")
    nc.vector.tensor_copy(x0i[:], iyx[:])   # round(v-0.5) == floor(v)
    x0f = pool.tile([NT, 2 * NTILES], F32, tag="x0f")
    nc.vector.tensor_copy(x0f[:], x0i[:])
    nc.vector.tensor_scalar(out=x0f[:], in0=x0f[:], scalar1=float(H - 2),
                            scalar2=0.0, op0=mybir.AluOpType.min,
                            op1=mybir.AluOpType.max)
    ys, xs = slice(0, NTILES), slice(NTILES, 2 * NTILES)
    idxf = pool.tile([NT, NTILES], F32, tag="idxf")
    nc.vector.tensor_scalar(out=idxf[:], in0=x0f[:, ys], scalar1=float(W),
                            scalar2=None, op0=MULT)
    nc.vector.tensor_tensor(out=idxf[:], in0=idxf[:], in1=x0f[:, xs], op=ADD)
    idxi = pool.tile([NT, NTILES], I32, tag="idxi")
    nc.vector.tensor_copy(idxi[:], idxf[:])
    frac = pool.tile([NT, 2 * NTILES], F32, tag="frac")
    nc.vector.tensor_tensor(out=frac[:], in0=iyx[:], in1=x0f[:], op=SUB)
    nc.vector.tensor_scalar_add(frac[:], frac[:], 0.5)
    omf = pool.tile([NT, 2 * NTILES], F32, tag="omf")
    nc.vector.tensor_scalar(out=omf[:], in0=frac[:], scalar1=-1.0, scalar2=1.0,
                            op0=MULT, op1=ADD)
    wts = [pool.tile([NT, NTILES], F32, tag=f"wts{j}", name=f"wts{j}") for j in range(4)]
    nc.vector.tensor_tensor(out=wts[0][:], in0=omf[:, ys], in1=omf[:, xs], op=MULT)
    nc.vector.tensor_tensor(out=wts[1][:], in0=omf[:, ys], in1=frac[:, xs], op=MULT)
    nc.vector.tensor_tensor(out=wts[2][:], in0=frac[:, ys], in1=omf[:, xs], op=MULT)
    nc.vector.tensor_tensor(out=wts[3][:], in0=frac[:, ys], in1=frac[:, xs], op=MULT)

    # ---------------- query conv (PE diag-matmuls, 392-pixel chunks) ----------
    diag_q = make_diag(wq, "q")
    q16 = [pool.tile([128, HALF_PIX], F16, tag=f"q_{ct}", name=f"q_{ct}")
           for ct in range(CT)]
    with tc.tile_pool(name="qps", bufs=2, space="PSUM") as qps:
        for ct in range(CT):
            x3 = xq16[ct][:].rearrange("p (h w) -> p h w", h=30)
            for chk in range(4):
                r0 = chk * 7
                ps = qps.tile([128, 392], F32, tag="qch", space="PSUM")
                for t in range(9):
                    dy, dx = t // 3, t % 3
                    src = x3[:, r0 + dy:r0 + dy + 7, dx:dx + W]
                    nc.tensor.matmul(ps[:].rearrange("p (h w) -> p h w", h=7),
                                     diag_q[ct][t], src,
                                     start=(t == 0), stop=(t == 8),
                                     skip_group_check=True)
                nc.scalar.activation(q16[ct][:, chk * 392:(chk + 1) * 392], ps[:],
                                     AF.Identity, bias=bq[ct][:, 0:1])

    # kT (PE transpose)
    kT = []
    with tc.tile_pool(name="ktp", bufs=3, space="PSUM") as ktp:
        for k in range(NTILES):
            t = pool.tile([NT, C], F16, tag=f"kT_{k}")
            for ct in range(CT):
                ps = ktp.tile([NT, 128], F16, tag="kt_ps", space="PSUM")
                nc.tensor.transpose(ps[:], key[ct][:, k * NT:(k + 1) * NT], ident[:])
                nc.scalar.activation(t[:, ct * 128:(ct + 1) * 128], ps[:], AF.Copy)
            kT.append(t)

    # ---------------- gathers + bilinear + M (PSUM-accumulated) ----------------
    vs = []
    with tc.tile_pool(name="m_psum", bufs=1, space="PSUM") as mps, \
         tc.tile_pool(name="gat", bufs=3) as gat:
        m_ps = [mps.tile([128, HC], F32, tag=f"m_ps{i}", name=f"m_ps{i}")
                for i in range(CT)]
        for k in range(NTILES):
            g0 = gat.tile([NT, 2 * C], F16, tag="g0")
            g1 = gat.tile([NT, 2 * C], F16, tag="g1")
            for g, delta in ((g0, 0), (g1, W)):
                gi = gp.indirect_dma_start(
                    out=g[:], out_offset=None, in_=io["vtab"][:, :],
                    in_offset=bass.IndirectOffsetOnAxis(ap=idxi[:, k:k + 1], axis=0),
                    element_offset=delta * C,
                    bounds_check=PIX - 1, oob_is_err=False)
                for wi in vtab_writes:
                    tile.add_dep_helper(gi.ins, wi.ins, reason="vtab RAW")
            v = pool.tile([NT, C], F16, tag=f"vs_{k}")
            nc.vector.tensor_scalar(out=v[:], in0=g0[:, 0:C], scalar1=wts[0][:, k:k + 1],
                                    scalar2=None, op0=MULT)
            nc.vector.scalar_tensor_tensor(out=v[:], in0=g0[:, C:2 * C],
                                           scalar=wts[1][:, k:k + 1], in1=v[:],
                                           op0=MULT, op1=ADD)
            nc.vector.scalar_tensor_tensor(out=v[:], in0=g1[:, 0:C],
                                           scalar=wts[2][:, k:k + 1], in1=v[:],
                                           op0=MULT, op1=ADD)
            nc.vector.scalar_tensor_tensor(out=v[:], in0=g1[:, C:2 * C],
                                           scalar=wts[3][:, k:k + 1], in1=v[:],
                                           op0=MULT, op1=ADD)
            vs.append(v)
            for h in range(NH):
                ct, j = h // 4, h % 4
                nc.tensor.matmul(m_ps[ct][j * 32:(j + 1) * 32, :],
                                 v[:, h * HC:(h + 1) * HC],
                                 kT[k][:, h * HC:(h + 1) * HC],
                                 start=(k == 0), stop=(k == NTILES - 1),
                                 tile_position=(0, j * 32),
                                 skip_group_check=True)
        m16 = []
        for ct in range(CT):
            t = pool.tile([128, HC], F16, tag=f"m16_{ct}")
            nc.scalar.activation(t[:], m_ps[ct][:], AF.Copy, scale=SCALE)
            m16.append(t)

    # ---------------- A^T = blockdiag(scale*M)^T w_out^T, then y ----------------
    at16 = []
    with tc.tile_pool(name="atps", bufs=1, space="PSUM") as atps:
        at_ps = [atps.tile([128, C], F32, tag=f"at_ps{i}", name=f"at_ps{i}")
                 for i in range(CT)]
        for h in range(NH):
            ct, j = h // 4, h % 4
            nc.tensor.matmul(at_ps[ct][j * 32:(j + 1) * 32, :],
                             m16[ct][j * 32:(j + 1) * 32, :],
                             wot16[ct][j * 32:(j + 1) * 32, :],
                             start=True, stop=True,
                             tile_position=(j * 32, j * 32))
        for ct in range(CT):
            t = pool.tile([128, C], F16, tag=f"at16_{ct}")
            nc.scalar.activation(t[:], at_ps[ct][:], AF.Copy)
            at16.append(t)

    NCHUNK = 4
    CW = HALF_PIX // NCHUNK  # 392
    with tc.tile_pool(name="yps", bufs=2, space="PSUM") as yps, \
         tc.tile_pool(name="ysb", bufs=3) as ysb:
        for ot in range(CT):
            for ch in range(NCHUNK):
                y_ps = yps.tile([128, CW], F32, tag="y_ps", space="PSUM")
                for ct in range(CT):
                    nc.tensor.matmul(y_ps[:], at16[ct][:, ot * 128:(ot + 1) * 128],
                                     q16[ct][:, ch * CW:(ch + 1) * CW],
                                     start=(ct == 0), stop=(ct == CT - 1))
                y_sb = ysb.tile([128, CW], F32, tag="y_sb")
                nc.scalar.activation(y_sb[:], y_ps[:], AF.Copy)
                dma2.dma_start(io["y"][ot * 128:(ot + 1) * 128, ch * CW:(ch + 1) * CW],
                              y_sb[:])


def build_program():
    if "nc" in _CACHE:
        return _CACHE["nc"]
    nc = bacc.Bacc("TRN2", target_bir_lowering=False, debug=False, num_devices=8)
    io = {}
    io["xp16"] = nc.dram_tensor("xp16", (C, HP * HP), F16, kind="ExternalInput").ap()
    io["xq"] = nc.dram_tensor("xq", (C, 30 * HP), F16, kind="ExternalInput").ap()
    for nm, shape in [("wv", (C, 9)), ("wq", (C, 9)), ("wk", (C, 9)), ("wo", (C, 9)),
                      ("bv", (C, 1)), ("bq", (C, 1)), ("bk", (C, 1)), ("bo", (C, 1)),
                      ("lng", (C, 1)), ("lnb", (C, 1)), ("w2t", (C, 2)),
                      ("refyx", (2, N))]:
        io[nm] = nc.dram_tensor(nm, shape, F32, kind="ExternalInput").ap()
    io["wot"] = nc.dram_tensor("wot", (C, C), F16, kind="ExternalInput").ap()
    io["vtab"] = nc.dram_tensor("vtab", (PIX, C), F16).ap()
    io["ixy_dram"] = nc.dram_tensor("ixy_dram", (2, N), F32).ap()
    io["y"] = nc.dram_tensor("y", (C, HALF_PIX), F32, kind="ExternalOutput").ap()

    with tile.TileContext(nc) as tc:
        with contextlib.ExitStack() as ctx:
            _emit(nc, tc, ctx, io)
    nc.compile()
    _CACHE["nc"] = nc
    return nc


def host_prep(inputs):
    """Build the 8 per-core input maps from full inputs."""
    x = np.asarray(inputs["x"], np.float32)          # (B, C, H, W)
    xpad = np.pad(x, ((0, 0), (0, 0), (1, 1), (1, 1)))  # (B, C, 58, 58)
    shared = {}
    for nm, src in [("wv", "w_v"), ("wq", "w_q"), ("wk", "w_k"), ("wo", "w_off1")]:
        shared[nm] = np.asarray(inputs[src], np.float32).reshape(C, 9)
    for nm, src in [("bv", "b_v"), ("bq", "b_q"), ("bk", "b_k"), ("bo", "b_off1"),
                    ("lng", "ln_g"), ("lnb", "ln_b")]:
        shared[nm] = np.asarray(inputs[src], np.float32).reshape(C, 1)
    shared["w2t"] = np.ascontiguousarray(np.asarray(inputs["w_off2"], np.float32).T)
    shared["wot"] = np.ascontiguousarray(
        np.asarray(inputs["w_out"], np.float32).T).astype(np.float16)   # (C,C) [c,o]
    ry = (np.arange(KH, dtype=np.float32) + 0.5) / KH * 2 - 1
    rx = (np.arange(KW, dtype=np.float32) + 0.5) / KW * 2 - 1
    refyx = np.stack([np.repeat(ry, KW), np.tile(rx, KH)])   # (2, 784), row0=y
    shared["refyx"] = np.ascontiguousarray(refyx, dtype=np.float32)

    in_maps = []
    for core in range(8):
        b, half = core // 2, core % 2
        m = dict(shared)
        xb = xpad[b]
        m["xp16"] = np.ascontiguousarray(xb.reshape(C, HP * HP)).astype(np.float16)
        r0 = half * HALF_ROWS
        m["xq"] = np.ascontiguousarray(
            xb[:, r0:r0 + 30, :].reshape(C, 30 * HP)).astype(np.float16)
        in_maps.append(m)
    return in_maps


def assemble(results):
    y = np.empty((B, C, H, W), np.float32)
    for core in range(8):
        b, half = core // 2, core % 2
        part = results[core]["y"].reshape(C, HALF_ROWS, W)
        y[b, :, half * HALF_ROWS:(half + 1) * HALF_ROWS, :] = part
    return y


def run(inputs, trace=False):
    nc = build_program()
    in_maps = host_prep(inputs)
    res = run_bass_kernel_spmd(nc, in_maps, core_ids=list(range(8)), trace=trace)
    return assemble(res.results), res


def kernel(**inputs):
    out, _ = run(inputs, trace=False)
    return out


# revision 27
# speedup vs baseline: 1.0898x; 1.0898x over previous
"""Trainium2 Bass kernel for nn_DeformableAttention (B=4, C=384, H=W=56, NH=12, HC=32, STRIDE=2).

Self-contained: hardcodes shapes/sharding. Sharding: 8 cores = 4 batches x 2
pixel-row-halves. Each core computes the full value/key/offset branches for its
batch (duplicated across the pair) and the query branch + final GEMM for its
half of the 3136 output pixels.

Math note: the reference computes out = (scale * q^T k) v^T without softmax, so
attention is linear and reassociates:
    y[b] = (w_out @ blockdiag_h(scale * M[b,h])) @ Q[b],
    M[b,h] = V_s[b,h] K[b,h]^T  (32x32 per head)
which drops the 48x(3136x784x32) einsums to a few small GEMMs.

Measured HW model (trace-derived): DVE ~1.1ns/elem regardless of dtype/stride;
PE matmul ~M+128 cycles; Pool elementwise ~2ns/elem with a one-time ~60us
ucode-load on the first op (pre-warmed with dummies); per-DMA-queue bandwidth
~90GB/s (big loads are split into slices to ride multiple queues).

Schedule: value conv runs on the PE as 9 diag(w_tap) matmuls PSUM-accumulated
per 448-pixel chunk (bias folded into the PSUM->SBUF copy), freeing ~92us of
DVE. The off branch is fp32 end-to-end (sample positions are precision
critical). DVE keeps off/key/query convs + LN pointwise + bilinear. Gathers
fetch (x0,x0+1) pixel pairs as one 768-elem row. M accumulates over the 7
k-tiles directly in PSUM. floor() is computed via round(x-0.5) (casts round to
nearest; integer ties land on the complementary-weight corner, which is exact).
"""
import contextlib

import numpy as np

import concourse.bass as bass
import concourse.tile as tile
from concourse import bacc, mybir
from concourse.bass_utils import run_bass_kernel_spmd
from concourse.masks import make_identity

F32, F16, I32 = mybir.dt.float32, mybir.dt.float16, mybir.dt.int32
MULT, ADD, SUB = mybir.AluOpType.mult, mybir.AluOpType.add, mybir.AluOpType.subtract
AF = mybir.ActivationFunctionType

B, C, H, W = 4, 384, 56, 56
NH, HC = 12, 32
SCALE = HC ** -0.5
HP = H + 2                      # 58 padded
PIX = H * W                     # 3136
KH = KW = 28                    # stride-2 output
N = KH * KW                     # 784
NT = 112                        # point-tile size (7 tiles)
NTILES = N // NT
HALF_ROWS = H // 2              # 28
HALF_PIX = HALF_ROWS * W        # 1568
CT = C // 128                   # 3 channel tiles
EPS = 1e-5
VCH = 448                       # value-conv PE chunk (8 rows of 56)
NVCH = PIX // VCH               # 7 chunks per ct

_CACHE = {}


def _emit(nc, tc, ctx, io):
    pool = ctx.enter_context(tc.tile_pool(name="main", bufs=1))
    dma = nc.sync
    gp = nc.gpsimd

    # ---------------- loads (big tensors split across DMA queues) ----------------
    def load_split(name, width, dtype, nsplit, eng):
        out = []
        for ct in range(CT):
            t = pool.tile([128, width], dtype, tag=f"{name}_{ct}")
            step = 128 // nsplit
            for s in range(nsplit):
                r0 = s * step
                eng.dma_start(t[r0:r0 + step, :],
                              io[name][ct * 128 + r0:ct * 128 + r0 + step, :])
            out.append(t)
        return out

    def load_cols(name, width, dtype=F32, eng=dma):
        out = []
        for ct in range(CT):
            t = pool.tile([128, width], dtype, tag=f"{name}_{ct}")
            eng.dma_start(t[:], io[name][ct * 128:(ct + 1) * 128, :])
            out.append(t)
        return out

    xp16 = load_split("xp16", HP * HP, F16, 4, dma)
    wv = load_cols("wv", 9)
    bv = load_cols("bv", 1)
    wo = load_cols("wo", 9)
    bo = load_cols("bo", 1)
    wk = load_cols("wk", 9)
    bk = load_cols("bk", 1)
    xq16 = load_split("xq", 30 * HP, F16, 2, dma)
    wq = load_cols("wq", 9)
    bq = load_cols("bq", 1)
    lng = load_cols("lng", 1)
    lnb = load_cols("lnb", 1)
    w2t32 = load_cols("w2t", 2)
    wot16 = load_cols("wot", C, dtype=F16, eng=dma)
    refyx = pool.tile([2, N], F32, tag="refyx")
    dma.dma_start(refyx[:], io["refyx"][:, :])
    ones_rc = pool.tile([128, 1], F16, tag="ones_rc")
    nc.vector.memset(ones_rc[:], 1.0 / C)
    one_row = pool.tile([1, 128], F16, tag="one_row")
    nc.vector.memset(one_row[:], 1.0)
    ident = pool.tile([128, 128], F16, tag="ident")
    make_identity(nc, ident[:])
    eps_t = pool.tile([1, 1], F32, tag="eps_t")
    nc.vector.memset(eps_t[:], EPS)

    # diag(w_tap) tiles (scalar engine: per-partition scale of the identity)
    def make_diag(w, nm):
        out = []
        for ct in range(CT):
            dd = []
            for t in range(9):
                d = pool.tile([128, 128], F16, tag=f"dg_{nm}_{ct}_{t}",
                              name=f"dg_{nm}_{ct}_{t}")
                nc.scalar.activation(d[:], ident[:], AF.Copy, scale=w[ct][:, t:t + 1])
                dd.append(d)
            out.append(dd)
        return out

    diag_v = make_diag(wv, "v")

    # ---------------- off conv (DVE, fp32 accumulate) ----------------
    off = []
    for ct in range(CT):
        t = pool.tile([128, N], F32, tag=f"off_{ct}")
        x3 = xp16[ct][:].rearrange("p (h w) -> p h w", h=HP)
        for tap in range(9):
            dy, dx = tap // 3, tap % 3
            src = x3[:, dy:dy + 2 * KH - 1:2, dx:dx + 2 * KW - 1:2]
            if tap == 0:
                nc.vector.tensor_scalar(out=t[:].rearrange("p (h w) -> p h w", h=KH),
                                        in0=src, scalar1=wo[ct][:, 0:1],
                                        scalar2=bo[ct][:, 0:1], op0=MULT, op1=ADD)
            else:
                nc.vector.scalar_tensor_tensor(
                    out=t[:].rearrange("p (h w) -> p h w", h=KH), in0=src,
                    scalar=wo[ct][:, tap:tap + 1],
                    in1=t[:].rearrange("p (h w) -> p h w", h=KH), op0=MULT, op1=ADD)
        off.append(t)

    # f16 copies for the LN-stats matmuls (stats precision is not critical)
    off16, sq16 = [], []
    for ct in range(CT):
        t = pool.tile([128, N], F16, tag=f"off16_{ct}")
        nc.scalar.activation(t[:], off[ct][:], AF.Copy)
        off16.append(t)
        s = pool.tile([128, N], F16, tag=f"sq16_{ct}")
        nc.scalar.activation(s[:], off[ct][:], AF.Square)
        sq16.append(s)

    # ---------------- value conv (PE diag-matmuls) + vtab transposes ----------
    val = [pool.tile([128, PIX], F16, tag=f"val_{ct}", name=f"val_{ct}")
           for ct in range(CT)]
    vps_ctx = tc.tile_pool(name="vps", bufs=2, space="PSUM")
    vps = vps_ctx.__enter__()
    vtctx = tc.tile_pool(name="vtp_ps", bufs=2, space="PSUM")
    vtps = vtctx.__enter__()
    vtsctx = tc.tile_pool(name="vtp_sb", bufs=3)
    vtsb = vtsctx.__enter__()
    vtab_writes = []

    def value_ct(ct):
        x3 = xp16[ct][:].rearrange("p (h w) -> p h w", h=HP)
        for chk in range(NVCH):
            r0 = chk * 8  # output row base of this 448-pixel chunk
            ps = vps.tile([128, VCH], F32, tag="vch", space="PSUM")
            for t in range(9):
                dy, dx = t // 3, t % 3
                src = x3[:, r0 + dy:r0 + dy + 8, dx:dx + W]
                nc.tensor.matmul(ps[:].rearrange("p (h w) -> p h w", h=8),
                                 diag_v[ct][t], src,
                                 start=(t == 0), stop=(t == 8),
                                 skip_group_check=True)
            nc.scalar.activation(val[ct][:, chk * VCH:(chk + 1) * VCH], ps[:],
                                 AF.Identity, bias=bv[ct][:, 0:1])

    def vtab_band(ct):
        for g in range(6):
            c0 = g * 512
            tp4 = vtps.tile([128, 512], F16, tag="tp4", space="PSUM")
            for j in range(4):
                nc.tensor.transpose(tp4[:, j * 128:(j + 1) * 128],
                                    val[ct][:, c0 + j * 128:c0 + (j + 1) * 128],
                                    ident[:])
            w4 = vtsb.tile([128, 512], F16, tag="w4")
            nc.scalar.activation(w4[:], tp4[:], AF.Copy)
            out_ap = bass.AP(io["vtab"].tensor, c0 * C + ct * 128,
                             [[C, 128], [C * 128, 4], [1, 128]])
            vtab_writes.append(dma.dma_start(out_ap, w4[:]))
        tp1 = vtps.tile([64, 128], F16, tag="tp1", space="PSUM")
        nc.tensor.transpose(tp1[:], val[ct][:, 3072:3136], ident[:])
        w1 = vtsb.tile([64, 128], F16, tag="w1")
        nc.scalar.activation(w1[:], tp1[:], AF.Copy)
        vtab_writes.append(
            dma.dma_start(io["vtab"][3072:3136, ct * 128:(ct + 1) * 128], w1[:]))

    value_ct(0)
    vtab_band(0)
    value_ct(1)
    vtab_band(1)

    # ---------------- LN stats (PE f16) + pointwise ----------------
    musq = pool.tile([1, N], F32, tag="musq")
    var = pool.tile([1, N], F32, tag="var")
    sd32 = pool.tile([1, N], F32, tag="sd32")
    rstd32 = pool.tile([1, N], F32, tag="rstd32")
    mu16 = pool.tile([1, N], F16, tag="mu16")
    rstd16 = pool.tile([1, N], F16, tag="rstd16")
    mu_b = pool.tile([128, N], F32, tag="mu_b")
    rstd_b = pool.tile([128, N], F32, tag="rstd_b")
    with tc.tile_pool(name="ln_psum", bufs=1, space="PSUM") as lnp:
        st_ps = lnp.tile([1, N], F32, tag="st_ps")
        for sl in (slice(0, 512), slice(512, N)):
            for ct in range(CT):
                nc.tensor.matmul(st_ps[:, sl], ones_rc[:], off16[ct][:, sl],
                                 start=(ct == 0), stop=(ct == CT - 1))
        nc.scalar.activation(musq[:], st_ps[:], AF.Square)
        nc.scalar.activation(mu16[:], st_ps[:], AF.Copy)
        for sl in (slice(0, 512), slice(512, N)):
            for ct in range(CT):
                nc.tensor.matmul(st_ps[:, sl], ones_rc[:], sq16[ct][:, sl],
                                 start=(ct == 0), stop=(ct == CT - 1))
        nc.vector.tensor_tensor(out=var[:], in0=st_ps[:], in1=musq[:], op=SUB)
    nc.scalar.activation(sd32[:], var[:], AF.Sqrt, bias=eps_t[:, 0:1])
    nc.vector.reciprocal_approx_fast(rstd32[:], sd32[:])
    nc.scalar.activation(rstd16[:], rstd32[:], AF.Copy)
    with tc.tile_pool(name="bc_psum", bufs=1, space="PSUM") as bcp:
        bc_ps = bcp.tile([128, N], F32, tag="bc_ps")
        for sl in (slice(0, 512), slice(512, N)):
            nc.tensor.matmul(bc_ps[:, sl], one_row[:], mu16[:, sl],
                             start=True, stop=True)
        nc.scalar.activation(mu_b[:], bc_ps[:], AF.Copy)
        for sl in (slice(0, 512), slice(512, N)):
            nc.tensor.matmul(bc_ps[:, sl], one_row[:], rstd16[:, sl],
                             start=True, stop=True)
        nc.scalar.activation(rstd_b[:], bc_ps[:], AF.Copy)

    # normalize (DVE, fp32) + gelu (scalar, g/b folded into activation)
    gel = []
    for ct in range(CT):
        t1 = off[ct]  # in-place
        nc.vector.tensor_tensor(out=t1[:], in0=t1[:], in1=mu_b[:], op=SUB)
        nc.vector.tensor_tensor(out=t1[:], in0=t1[:], in1=rstd_b[:], op=MULT)
        g = pool.tile([128, N], F32, tag=f"gel_{ct}")
        nc.scalar.activation(g[:], t1[:], AF.Gelu,
                             scale=lng[ct][:, 0:1], bias=lnb[ct][:, 0:1])
        gel.append(g)

    # ---------------- key conv (DVE, strided taps) ----------------
    key = []
    for ct in range(CT):
        t = pool.tile([128, N], F16, tag=f"key_{ct}")
        x3 = xp16[ct][:].rearrange("p (h w) -> p h w", h=HP)
        for tap in range(9):
            dy, dx = tap // 3, tap % 3
            src2 = x3[:, dy:dy + 2 * KH - 1:2, dx:dx + 2 * KW - 1:2]
            o2 = t[:].rearrange("p (h w) -> p h w", h=KH)
            if tap == 0:
                nc.vector.tensor_scalar(out=o2, in0=src2, scalar1=wk[ct][:, 0:1],
                                        scalar2=bk[ct][:, 0:1], op0=MULT, op1=ADD)
            else:
                nc.vector.scalar_tensor_tensor(out=o2, in0=src2,
                                               scalar=wk[ct][:, tap:tap + 1],
                                               in1=o2, op0=MULT, op1=ADD)
        key.append(t)


    value_ct(2)
    vtab_band(2)
    vtsctx.__exit__(None, None, None)
    vtctx.__exit__(None, None, None)
    vps_ctx.__exit__(None, None, None)

    # ---------------- offset head: w2t matmul (fp32) + tanh + ixy ----------------
    pos = pool.tile([2, N], F32, tag="pos")
    tnh = pool.tile([2, N], F32, tag="tnh")
    ixy0 = pool.tile([2, N], F32, tag="ixy0")
    with tc.tile_pool(name="off_psum", bufs=1, space="PSUM") as offp:
        oyx_ps = offp.tile([2, N], F32, tag="oyx")
        for sl in (slice(0, 512), slice(512, N)):
            for ct in range(CT):
                nc.tensor.matmul(oyx_ps[:, sl], w2t32[ct][:], gel[ct][:, sl],
                                 start=(ct == 0), stop=(ct == CT - 1))
        oyx_sb = pool.tile([2, N], F32, tag="oyx_sb")
        nc.scalar.activation(oyx_sb[:], oyx_ps[:], AF.Copy)
    nc.vector.tensor_tensor(out=pos[:], in0=oyx_sb[:], in1=refyx[:], op=ADD)
    nc.scalar.activation(tnh[:], pos[:], AF.Tanh)
    # iy/ix - 0.5 = tanh*27.5 + 27.0  (the -0.5 shift makes round() act as floor)
    nc.vector.tensor_scalar(out=ixy0[:], in0=tnh[:], scalar1=(H - 1) / 2.0,
                            scalar2=(H - 1) / 2.0 - 0.5, op0=MULT, op1=ADD)
    ixy_write = dma.dma_start(io["ixy_dram"][:, :], ixy0[:])

    # ---------------- index math (DVE, wide tiles) ----------------
    # layout [112 pts, 14]: cols 0..6 = iy-0.5 per k-tile, cols 7..13 = ix-0.5
    iyx = pool.tile([NT, 2 * NTILES], F32, tag="iyx")
    for j in range(2):
        src = bass.AP(io["ixy_dram"].tensor, j * N, [[1, NT], [NT, NTILES]])
        rd = dma.dma_start(iyx[:, j * NTILES:(j + 1) * NTILES], src)
        tile.add_dep_helper(rd.ins, ixy_write.ins, reason="ixy dram RAW")
    x0i = pool.tile([NT, 2 * NTILES], I32, tag="x0i")
    nc.vector.tensor_copy(x0i[:], iyx[:])   # round(v-0.5) == floor(v)
    x0f = pool.tile([NT, 2 * NTILES], F32, tag="x0f")
    nc.vector.tensor_copy(x0f[:], x0i[:])
    nc.vector.tensor_scalar(out=x0f[:], in0=x0f[:], scalar1=float(H - 2),
                            scalar2=0.0, op0=mybir.AluOpType.min,
                            op1=mybir.AluOpType.max)
    ys, xs = slice(0, NTILES), slice(NTILES, 2 * NTILES)
    idxf = pool.tile([NT, NTILES], F32, tag="idxf")
    nc.vector.tensor_scalar(out=idxf[:], in0=x0f[:, ys], scalar1=float(W),
                            scalar2=None, op0=MULT)
    nc.vector.tensor_tensor(out=idxf[:], in0=idxf[:], in1=x0f[:, xs], op=ADD)
    idxi = pool.tile([NT, NTILES], I32, tag="idxi")
    nc.vector.tensor_copy(idxi[:], idxf[:])
    frac = pool.tile([NT, 2 * NTILES], F32, tag="frac")
    nc.vector.tensor_tensor(out=frac[:], in0=iyx[:], in1=x0f[:], op=SUB)
    nc.vector.tensor_scalar_add(frac[:], frac[:], 0.5)
    omf = pool.tile([NT, 2 * NTILES], F32, tag="omf")
    nc.vector.tensor_scalar(out=omf[:], in0=frac[:], scalar1=-1.0, scalar2=1.0,
                            op0=MULT, op1=ADD)
    wts = [pool.tile([NT, NTILES], F32, tag=f"wts{j}", name=f"wts{j}") for j in range(4)]
    nc.vector.tensor_tensor(out=wts[0][:], in0=omf[:, ys], in1=omf[:, xs], op=MULT)
    nc.vector.tensor_tensor(out=wts[1][:], in0=omf[:, ys], in1=frac[:, xs], op=MULT)
    nc.vector.tensor_tensor(out=wts[2][:], in0=frac[:, ys], in1=omf[:, xs], op=MULT)
    nc.vector.tensor_tensor(out=wts[3][:], in0=frac[:, ys], in1=frac[:, xs], op=MULT)

    # ---------------- query conv (PE diag-matmuls, 392-pixel chunks) ----------
    diag_q = make_diag(wq, "q")
    q16 = [pool.tile([128, HALF_PIX], F16, tag=f"q_{ct}", name=f"q_{ct}")
           for ct in range(CT)]
    with tc.tile_pool(name="qps", bufs=2, space="PSUM") as qps:
        for ct in range(CT):
            x3 = xq16[ct][:].rearrange("p (h w) -> p h w", h=30)
            for chk in range(4):
                r0 = chk * 7
                ps = qps.tile([128, 392], F32, tag="qch", space="PSUM")
                for t in range(9):
                    dy, dx = t // 3, t % 3
                    src = x3[:, r0 + dy:r0 + dy + 7, dx:dx + W]
                    nc.tensor.matmul(ps[:].rearrange("p (h w) -> p h w", h=7),
                                     diag_q[ct][t], src,
                                     start=(t == 0), stop=(t == 8),
                                     skip_group_check=True)
                nc.scalar.activation(q16[ct][:, chk * 392:(chk + 1) * 392], ps[:],
                                     AF.Identity, bias=bq[ct][:, 0:1])

    # kT (PE transpose)
    kT = []
    with tc.tile_pool(name="ktp", bufs=3, space="PSUM") as ktp:
        for k in range(NTILES):
            t = pool.tile([NT, C], F16, tag=f"kT_{k}")
            for ct in range(CT):
                ps = ktp.tile([NT, 128], F16, tag="kt_ps", space="PSUM")
                nc.tensor.transpose(ps[:], key[ct][:, k * NT:(k + 1) * NT], ident[:])
                nc.scalar.activation(t[:, ct * 128:(ct + 1) * 128], ps[:], AF.Copy)
            kT.append(t)

    # ---------------- gathers + bilinear + M (PSUM-accumulated) ----------------
    vs = []
    with tc.tile_pool(name="m_psum", bufs=1, space="PSUM") as mps, \
         tc.tile_pool(name="gat", bufs=3) as gat:
        m_ps = [mps.tile([128, HC], F32, tag=f"m_ps{i}", name=f"m_ps{i}")
                for i in range(CT)]
        for k in range(NTILES):
            g0 = gat.tile([NT, 2 * C], F16, tag="g0")
            g1 = gat.tile([NT, 2 * C], F16, tag="g1")
            for g, delta in ((g0, 0), (g1, W)):
                gi = gp.indirect_dma_start(
                    out=g[:], out_offset=None, in_=io["vtab"][:, :],
                    in_offset=bass.IndirectOffsetOnAxis(ap=idxi[:, k:k + 1], axis=0),
                    element_offset=delta * C,
                    bounds_check=PIX - 1, oob_is_err=False)
                for wi in vtab_writes:
                    tile.add_dep_helper(gi.ins, wi.ins, reason="vtab RAW")
            v = pool.tile([NT, C], F16, tag=f"vs_{k}")
            nc.vector.tensor_scalar(out=v[:], in0=g0[:, 0:C], scalar1=wts[0][:, k:k + 1],
                                    scalar2=None, op0=MULT)
            nc.vector.scalar_tensor_tensor(out=v[:], in0=g0[:, C:2 * C],
                                           scalar=wts[1][:, k:k + 1], in1=v[:],
                                           op0=MULT, op1=ADD)
            nc.vector.scalar_tensor_tensor(out=v[:], in0=g1[:, 0:C],
                                           scalar=wts[2][:, k:k + 1], in1=v[:],
                                           op0=MULT, op1=ADD)
            nc.vector.scalar_tensor_tensor(out=v[:], in0=g1[:, C:2 * C],
                                           scalar=wts[3][:, k:k + 1], in1=v[:],
                                           op0=MULT, op1=ADD)
            vs.append(v)
            for h in range(NH):
                ct, j = h // 4, h % 4
                nc.tensor.matmul(m_ps[ct][j * 32:(j + 1) * 32, :],
                                 v[:, h * HC:(h + 1) * HC],
                                 kT[k][:, h * HC:(h + 1) * HC],
                                 start=(k == 0), stop=(k == NTILES - 1),
                                 tile_position=(0, j * 32),
                                 skip_group_check=True)
        m16 = []
        for ct in range(CT):
            t = pool.tile([128, HC], F16, tag=f"m16_{ct}")
            nc.scalar.activation(t[:], m_ps[ct][:], AF.Copy, scale=SCALE)
            m16.append(t)

    # ---------------- A^T = blockdiag(scale*M)^T w_out^T, then y ----------------
    at16 = []
    with tc.tile_pool(name="atps", bufs=1, space="PSUM") as atps:
        at_ps = [atps.tile([128, C], F32, tag=f"at_ps{i}", name=f"at_ps{i}")
                 for i in range(CT)]
        for h in range(NH):
            ct, j = h // 4, h % 4
            nc.tensor.matmul(at_ps[ct][j * 32:(j + 1) * 32, :],
                             m16[ct][j * 32:(j + 1) * 32, :],
                             wot16[ct][j * 32:(j + 1) * 32, :],
                             start=True, stop=True,
                             tile_position=(j * 32, j * 32))
        for ct in range(CT):
            t = pool.tile([128, C], F16, tag=f"at16_{ct}")
            nc.scalar.activation(t[:], at_ps[ct][:], AF.Copy)
            at16.append(t)

    NCHUNK = 4
    CW = HALF_PIX // NCHUNK  # 392
    with tc.tile_pool(name="yps", bufs=2, space="PSUM") as yps, \
         tc.tile_pool(name="ysb", bufs=3) as ysb:
        for ot in range(CT):
            for ch in range(NCHUNK):
                y_ps = yps.tile([128, CW], F32, tag="y_ps", space="PSUM")
                for ct in range(CT):
                    nc.tensor.matmul(y_ps[:], at16[ct][:, ot * 128:(ot + 1) * 128],
                                     q16[ct][:, ch * CW:(ch + 1) * CW],
                                     start=(ct == 0), stop=(ct == CT - 1))
                y_sb = ysb.tile([128, CW], F32, tag="y_sb")
                nc.scalar.activation(y_sb[:], y_ps[:], AF.Copy)
                dma.dma_start(io["y"][ot * 128:(ot + 1) * 128, ch * CW:(ch + 1) * CW],
                              y_sb[:])


def build_program():
    if "nc" in _CACHE:
        return _CACHE["nc"]
    nc = bacc.Bacc("TRN2", target_bir_lowering=False, debug=False, num_devices=8)
    io = {}
    io["xp16"] = nc.dram_tensor("xp16", (C, HP * HP), F16, kind="ExternalInput").ap()
    io["xq"] = nc.dram_tensor("xq", (C, 30 * HP), F16, kind="ExternalInput").ap()
    for nm, shape in [("wv", (C, 9)), ("wq", (C, 9)), ("wk", (C, 9)), ("wo", (C, 9)),
                      ("bv", (C, 1)), ("bq", (C, 1)), ("bk", (C, 1)), ("bo", (C, 1)),
                      ("lng", (C, 1)), ("lnb", (C, 1)), ("w2t", (C, 2)),
                      ("refyx", (2, N))]:
        io[nm] = nc.dram_tensor(nm, shape, F32, kind="ExternalInput").ap()
    io["wot"] = nc.dram_tensor("wot", (C, C), F16, kind="ExternalInput").ap()
    io["vtab"] = nc.dram_tensor("vtab", (PIX, C), F16).ap()
    io["ixy_dram"] = nc.dram_tensor("ixy_dram", (2, N), F32).ap()
    io["y"] = nc.dram_tensor("y", (C, HALF_PIX), F32, kind="ExternalOutput").ap()

    with tile.TileContext(nc) as tc:
        with contextlib.ExitStack() as ctx:
            _emit(nc, tc, ctx, io)
    nc.compile()
    _CACHE["nc"] = nc
    return nc


def host_prep(inputs):
    """Build the 8 per-core input maps from full inputs."""
    x = np.asarray(inputs["x"], np.float32)          # (B, C, H, W)
    xpad = np.pad(x, ((0, 0), (0, 0), (1, 1), (1, 1)))  # (B, C, 58, 58)
    shared = {}
    for nm, src in [("wv", "w_v"), ("wq", "w_q"), ("wk", "w_k"), ("wo", "w_off1")]:
        shared[nm] = np.asarray(inputs[src], np.float32).reshape(C, 9)
    for nm, src in [("bv", "b_v"), ("bq", "b_q"), ("bk", "b_k"), ("bo", "b_off1"),
                    ("lng", "ln_g"), ("lnb", "ln_b")]:
        shared[nm] = np.asarray(inputs[src], np.float32).reshape(C, 1)
    shared["w2t"] = np.ascontiguousarray(np.asarray(inputs["w_off2"], np.float32).T)
    shared["wot"] = np.ascontiguousarray(
        np.asarray(inputs["w_out"], np.float32).T).astype(np.float16)   # (C,C) [c,o]
    ry = (np.arange(KH, dtype=np.float32) + 0.5) / KH * 2 - 1
    rx = (np.arange(KW, dtype=np.float32) + 0.5) / KW * 2 - 1
    refyx = np.stack([np.repeat(ry, KW), np.tile(rx, KH)])   # (2, 784), row0=y
    shared["refyx"] = np.ascontiguousarray(refyx, dtype=np.float32)

    in_maps = []
    for core in range(8):
        b, half = core // 2, core % 2
        m = dict(shared)
        xb = xpad[b]
        m["xp16"] = np.ascontiguousarray(xb.reshape(C, HP * HP)).astype(np.float16)
        r0 = half * HALF_ROWS
        m["xq"] = np.ascontiguousarray(
            xb[:, r0:r0 + 30, :].reshape(C, 30 * HP)).astype(np.float16)
        in_maps.append(m)
    return in_maps


def assemble(results):
    y = np.empty((B, C, H, W), np.float32)
    for core in range(8):
        b, half = core // 2, core % 2
        part = results[core]["y"].reshape(C, HALF_ROWS, W)
        y[b, :, half * HALF_ROWS:(half + 1) * HALF_ROWS, :] = part
    return y


def run(inputs, trace=False):
    nc = build_program()
    in_maps = host_prep(inputs)
    res = run_bass_kernel_spmd(nc, in_maps, core_ids=list(range(8)), trace=trace)
    return assemble(res.results), res


def kernel(**inputs):
    out, _ = run(inputs, trace=False)
    return out
